# revision 26
# baseline (speedup 1.0000x reference)
"""Multi-head attention on 8 Trainium2 NeuronCores.

Sharding: core c = (batch n, head-group g); n = c // 4, g = c % 4.
Each core computes attention for its 4 heads of its batch entry plus the
fc_out partial product for those heads' columns of Wo; the host sums the
4 partials per batch (and adds the bias) to unshard.

Pipeline (all matmuls bf16, f32 PSUM accumulation). ScalarE exp is the
roofline (~147us: 128 x [128,1024] activations at (N+352)/1.2 ns); the
whole schedule is built to start that stream early and never starve it:

  A) Minimal head start: pair-0 q/k projection for q-columns 0-1023
     (lh0) only, then the exp-paced main loop begins. Everything else
     (lh1, pair-1 q/k, v, fc lcp=0) is emitted as filler batches
     between steps.
  B) 64 steps (4 blocks (pair,qs) x 16 k-tiles). Per step: scores for
     step s+1 (2 sides x 2 halves, [64,128] row-tiled), exp A + exp B
     ([128,1024] each, psS bufs=2 gives a side-staggered double
     buffer: A's banks recycle under B's exp), then attn@v for step s:
     col-tiled pairs - head A -> PSUM partitions 0-63, head B -> 64-127
     run concurrently (M=64 each), landing directly in the outTP
     partition layout (no post-hoc shift DMA). The softmax denominator
     comes from DVE partial sums of the exp tiles (bf16 tensor_tensor,
     2x mode) reduced across partitions by a col-tiled ones-matmul
     whose M=64 replication IS the broadcast for the normalize multiply
     (no DRAM round-trips).
  C) fc_out: lcp=0 chunks run as fillers once qs=0 blocks are
     normalized; lcp=1 is the tail, evacuations alternating
     ScalarE/VectorE, output shipped in 512KB pieces.
"""

import contextlib as _contextlib
import os
import sys

for _p in ("/opt/trn_rl_repo",):
    if _p not in sys.path and os.path.isdir(_p):
        sys.path.insert(0, _p)

import numpy as np
import ml_dtypes

import concourse.bass as bass
import concourse.mybir as mybir
import concourse.tile as tile
from concourse import bacc
from concourse.bass import ds, ts
from concourse.bass_utils import run_bass_kernel_spmd

BF16 = ml_dtypes.bfloat16
F32 = np.float32

EMBED = 1024
HEADS = 16
HD = 64  # head dim
NB = 2  # batch
L = 2048  # sequence length
NCORES = 8
HPG = 4  # heads per core (group)
NPAIRS = 2  # head pairs per core
ET = EMBED // 128  # 8 contraction tiles for projections
LT = L // 128  # 16 k tiles

SCALE = 1.0 / np.sqrt(np.float32(EMBED))  # 1/32

# block order: qs=0 blocks first so fc lcp=0 can run as filler work
BLOCKS = [(0, 0), (1, 0), (0, 1), (1, 1)]  # (pair, qs)
NSTEPS = len(BLOCKS) * LT  # 64

LAST_EXEC_TIME_NS = None
LAST_RESULTS = None

_nc_cache = None


def build_nc():
    """Build + compile the per-core Bass program (same program on all cores)."""
    nc = bacc.Bacc("TRN2")
    f32 = mybir.dt.float32
    bf16 = mybir.dt.bfloat16
    EXP = mybir.ActivationFunctionType.Exp

    xT_d = nc.declare_dram_parameter("xT", [EMBED, L], bf16, isOutput=False)
    # weights arrive partition-major from the host so every DMA row is a
    # contiguous 2-8KB run (256B rows serialize the DGE and gate startup)
    wqk_d = nc.declare_dram_parameter("wqk", [128, 4, ET, 128], bf16, isOutput=False)
    wv_d = nc.declare_dram_parameter("wv", [128, ET, HPG * HD], bf16, isOutput=False)
    wo_d = nc.declare_dram_parameter("wo", [128, NPAIRS, ET, 128], bf16, isOutput=False)
    out_d = nc.declare_dram_parameter("out", [EMBED, L], bf16, isOutput=True)

    with tile.TileContext(nc) as tc:
        with (
            tc.tile_pool(name="singles", bufs=1) as singles,
            tc.tile_pool(name="expp", bufs=18) as expp,
            tc.tile_pool(name="pap", bufs=2) as pap,
            tc.tile_pool(name="rcp", bufs=3) as rcp,
        ):
            # ---- resident SBUF tensors ----
            xT_sb = singles.tile([128, ET, L], bf16, name="xT_sb")
            wqk_sb = singles.tile([128, 4, ET, 128], bf16, name="wqk_sb")
            wv_sb = singles.tile([128, ET, HPG * HD], bf16, name="wv_sb")
            wo_sb = singles.tile([128, NPAIRS, ET, 128], bf16, name="wo_sb")
            qt_sb = singles.tile([128, NPAIRS, L], bf16, name="qt_sb")
            kt_sb = singles.tile([128, NPAIRS, L], bf16, name="kt_sb")
            v_sb = singles.tile([128, LT, HPG, HD], bf16, name="v_sb")
            outTP_sb = singles.tile([128, NPAIRS, L], bf16, name="outTP_sb")
            ones_sb = singles.tile([128, HD], bf16, name="ones_sb")
            warm_sb = singles.tile([128, 512], bf16, name="warm_sb")

            # ---- input DMAs, ONE queue, strict priority order ----
            xT_ap = xT_d[:].rearrange("(t p) l -> p t l", p=128)
            for j in range(2):
                nc.sync.dma_start(out=wqk_sb[:, j, :, :], in_=wqk_d[:, j, :, :])
            for et in range(ET):
                nc.sync.dma_start(
                    out=xT_sb[:, et, ts(0, 1024)], in_=xT_ap[:, et, ts(0, 1024)]
                )
            nc.sync.dma_start(out=wv_sb, in_=wv_d[:])
            for et in range(ET):
                nc.sync.dma_start(
                    out=xT_sb[:, et, ts(1, 1024)], in_=xT_ap[:, et, ts(1, 1024)]
                )
            for j in range(2, 4):
                nc.sync.dma_start(out=wqk_sb[:, j, :, :], in_=wqk_d[:, j, :, :])
            nc.sync.dma_start(out=wo_sb, in_=wo_d[:])

            # preload the exp spline tables during the DMA-bound window
            nc.vector.memset(ones_sb, 1.0)
            nc.vector.memset(warm_sb, 1.0)
            nc.scalar.activation(ones_sb[0:1, 0:2], warm_sb[0:1, 0:2], EXP)

            # filler psum (2 banks) opened FIRST so psS can close before
            # it at the tail, freeing 4 banks for the wide fc pool psC
            _psA2_stack = _contextlib.ExitStack()
            psA2 = _psA2_stack.enter_context(
                tc.tile_pool(name="psA2", bufs=2, space="PSUM")
            )
            _psS_stack = _contextlib.ExitStack()
            psS = _psS_stack.enter_context(
                tc.tile_pool(name="psS", bufs=2, space="PSUM")
            )

            # warm the PE from t~7us on memset data (no DMA dependency):
            # HAM ramps to full clock before the first projection arrives
            warm = psS.tile([128, 512], f32, tag="sc", name="warm0")
            for _ in range(12):
                nc.tensor.matmul(
                    warm,
                    warm_sb[:, 0:128],
                    warm_sb,
                    start=True,
                    stop=True,
                )

            # ================= lh0: pair-0 q/k, columns 0-1023 ========
            # accumulators borrow psA2's 2 bufs + 2 of psS's (the warm
            # tile rotates out) - no dedicated pool, 6 banks total live
            pst = [
                psA2.tile([128, 512], f32, tag="ps2", name="qk0_0"),
                psA2.tile([128, 512], f32, tag="ps2", name="qk0_1"),
                psS.tile([128, 512], f32, tag="sc", name="qk0_2"),
                psS.tile([128, 512], f32, tag="sc", name="qk0_3"),
            ]
            for et in range(ET):
                for j in range(2):
                    for l2 in range(2):
                        nc.tensor.matmul(
                            pst[j * 2 + l2],
                            wqk_sb[:, j, et, :],
                            xT_sb[:, et, ts(l2, 512)],
                            start=(et == 0),
                            stop=(et == ET - 1),
                        )
            # evacs split across ScalarE (idle pre-stream) and VectorE;
            # scores(0) needs only kt cols 0-127 + qt cols 0-1023, so a
            # narrow kt slice goes first and the rest follows
            nc.vector.tensor_copy(kt_sb[:, 0, 0:128], pst[2][:, 0:128])
            nc.scalar.copy(qt_sb[:, 0, ts(0, 512)], pst[0])
            nc.vector.tensor_copy(qt_sb[:, 0, ts(1, 512)], pst[1])
            nc.scalar.copy(kt_sb[:, 0, ds(128, 384)], pst[2][:, 128:512])
            nc.vector.tensor_copy(kt_sb[:, 0, ts(1, 512)], pst[3])

            # ---- filler pieces: <=0.9us of PE work each --------------
            def f_vchunk(lt):
                def go():
                    pv = psA2.tile([128, 512], f32, tag="ps2", name=f"v{lt}")
                    pv = pv[:, : HPG * HD]
                    for et in range(ET):
                        nc.tensor.matmul(
                            pv,
                            xT_sb[:, et, ts(lt, 128)],
                            wv_sb[:, et, :],
                            start=(et == 0),
                            stop=(et == ET - 1),
                        )
                    nc.vector.tensor_copy(
                        v_sb[:, lt, :, :],
                        pv.rearrange("p (h d) -> p h d", h=HPG),
                    )
                return go

            def f_qkpiece(j, col0):
                # one 256-column slice of a q/k projection: 8 N=256
                # matmuls + a [128,256] evac (~0.9us PE, independent)
                def go():
                    pq = psA2.tile([128, 512], f32, tag="ps2", name=f"qk{j}_{col0}")
                    pq = pq[:, 0:256]
                    for et in range(ET):
                        nc.tensor.matmul(
                            pq,
                            wqk_sb[:, j, et, :],
                            xT_sb[:, et, ds(col0, 256)],
                            start=(et == 0),
                            stop=(et == ET - 1),
                        )
                    dst = qt_sb if j % 2 == 0 else kt_sb
                    nc.vector.tensor_copy(dst[:, j // 2, ds(col0, 256)], pq)
                return go

            out_ap = out_d[:].rearrange("(t p) l -> p t l", p=128)

            fc_ps = {}

            def f_fcpiece(et, half, pair):
                # streamed fc lcp=0: one pair-partial matmul (~270ns of
                # PE); the [128,512] psum is held across the two pieces,
                # DVE evac into the dead xT staging area, per-2-et ship
                def go():
                    if pair == 0:
                        fc_ps[(et, half)] = psA2.tile(
                            [128, 512], f32, tag="ps2", name=f"f{et}{half}"
                        )
                    fp = fc_ps[(et, half)]
                    nc.tensor.matmul(
                        fp,
                        wo_sb[:, pair, et, :],
                        outTP_sb[:, pair, ts(half, 512)],
                        start=(pair == 0),
                        stop=(pair == NPAIRS - 1),
                    )
                    if pair == NPAIRS - 1:
                        nc.vector.tensor_copy(
                            xT_sb[:, et, ds(half * 512, 512)], fp
                        )
                        del fc_ps[(et, half)]
                        if half == 1 and et % 2 == 1:
                            eh = et // 2
                            nc.sync.dma_start(
                                out=out_ap[:, ts(eh, 2), ts(0, 1024)],
                                in_=xT_sb[:, ts(eh, 2), ts(0, 1024)],
                            )
                return go

            # (due_step, v_credit, fn), sorted.  The stream is PE-bound,
            # so dues only encode hard dependency deadlines: kt pair-0
            # cols 1024+256c before scores k-tile 8+2c at step 7+2c;
            # pair-1 q/k before block-1 scores from step 15; qs=1 q
            # columns before block-2 scores from step 31; fc lcp=0 after
            # block-1 normalize.  attn@v pops are gated on the v chunks
            # actually emitted (v_credit), so a late v never stalls the
            # PE - the ex pool absorbs the lag.
            fillers = []
            for lt in range(LT):
                due = lt if lt < 8 else 9 + 2 * (lt - 8)
                fillers.append((due, 1, f_vchunk(lt)))
            for c in range(4):  # lh1 kt: cols 1024+256c
                fillers.append((2 + c, 0, f_qkpiece(1, 1024 + 256 * c)))
            for c in range(4):  # pair-1 q, qs=0
                fillers.append((7 + c, 0, f_qkpiece(2, 256 * c)))
            for c in range(8):  # pair-1 k: cols 256c
                fillers.append((11 + c, 0, f_qkpiece(3, 256 * c)))
            for i, c in enumerate((24, 26, 28, 30)):  # lh1 q, qs=1
                fillers.append((c, 0, f_qkpiece(0, 1024 + 256 * i)))
            for i, c in enumerate((33, 35, 37, 39)):  # pair-1 q, qs=1
                fillers.append((c, 0, f_qkpiece(2, 1024 + 256 * i)))
            fc_due = [d for d in range(41, 63) if d not in (47, 48)]
            fc_i = 0
            for et in range(ET):  # fc lcp=0, two pieces per step
                for half in range(2):
                    for pair in range(NPAIRS):
                        fillers.append(
                            (fc_due[fc_i // 2], 0, f_fcpiece(et, half, pair))
                        )
                        fc_i += 1
            fillers.sort(key=lambda p: p[0])

            def pop_due(s):
                while fillers and fillers[0][0] <= s:
                    fillers.pop(0)[1]()

            # ================= main exp-paced loop ====================
            # attn@v psum: 2 banks, single-buffered per block (the next
            # block's start=True write waits on this block's normalize;
            # av is not on ScalarE's critical path)
            _psB_stack = _contextlib.ExitStack()
            psB = _psB_stack.enter_context(
                tc.tile_pool(name="psB", bufs=2, space="PSUM")
            )

            def emit_scores(s):
                blk, k = divmod(s, LT)
                pair, qs = BLOCKS[blk]
                sides = []
                for side in range(2):
                    st = psS.tile([128, 1024], f32, tag="sc", name=f"sc{side}")
                    base = side * HD
                    for half in range(2):
                        nc.tensor.matmul(
                            st[:, ts(half, 512)],
                            kt_sb[base : base + HD, pair, ts(k, 128)],
                            qt_sb[
                                base : base + HD,
                                pair,
                                ds(qs * 1024 + half * 512, 512),
                            ],
                            start=True,
                            stop=True,
                        )
                    sides.append(st)
                return sides

            def emit_exp(sides):
                ex = expp.tile([128, 2048], bf16, tag="exp", name="ex")
                for side in range(2):
                    nc.scalar.activation(
                        ex[:, ds(side * 1024, 1024)],
                        sides[side],
                        EXP,
                        scale=float(SCALE),
                    )
                return ex

            av_ps = {}
            pa_of = {}

            def emit_av(s, ex):
                blk, k = divmod(s, LT)
                pair, qs = BLOCKS[blk]
                if k == 0:
                    av_ps[blk] = [
                        psB.tile([128, 512], f32, tag="av", name=f"av{blk}{h}")
                        for h in range(2)
                    ]
                av = av_ps[blk]
                # col-tiled pairs: head A -> PSUM partitions 0-63 (tile
                # (0,0)), head B -> 64-127 (tile (0,64)).  Order
                # A_h0, B_h0, B_h1, A_h1: consecutive MMs alternate col
                # tiles (concurrent streams) AND reuse resident weights
                for half, side in ((0, 0), (0, 1), (1, 1), (1, 0)):
                    nc.tensor.matmul(
                        av[half][side * HD : (side + 1) * HD, :],
                        v_sb[:, k, pair * 2 + side, :],
                        ex[:, ds(side * 1024 + half * 512, 512)],
                        start=(k == 0),
                        stop=(k == LT - 1),
                    )

            def emit_pa(s, ex):
                blk, k = divmod(s, LT)
                if k == 0:
                    pa_of[blk] = pap.tile([128, 2048], bf16, tag="pa", name="pa")
                    nc.vector.tensor_copy(pa_of[blk], ex)
                else:
                    nc.vector.tensor_add(pa_of[blk], pa_of[blk], ex)

            def emit_norm(blk):
                # denominator: col-tiled ones-matmul partition-reduce of
                # the exp partial sums; the M=64 replication doubles as
                # the broadcast.  reciprocal + normalize straight out of
                # the av PSUM into outTP.
                pair, qs = BLOCKS[blk]
                pa = pa_of.pop(blk)
                av = av_ps.pop(blk)
                for half in range(2):
                    den = psA2.tile([128, 512], f32, tag="ps2", name=f"dn{half}")
                    for side in range(2):
                        nc.tensor.matmul(
                            den[side * HD : (side + 1) * HD, :],
                            ones_sb,
                            pa[:, ds(side * 1024 + half * 512, 512)],
                            start=True,
                            stop=True,
                        )
                    rc = rcp.tile([128, 512], f32, tag="rc", name="rc")
                    nc.vector.reciprocal_approx_fast(rc, den)
                    nc.vector.tensor_mul(
                        outTP_sb[:, pair, ds(qs * 1024 + half * 512, 512)],
                        av[half],
                        rc,
                    )

            # attn@v trails the exp stream by LAG steps so the early
            # steps have PE room for the projection fillers; pops are
            # gated on the v chunks being emitted (PE never stalls on a
            # missing v) and catch back up to lag 1 from step 40.
            LAG = 8
            av_q = []
            v_done = [0]

            def pop_avs(s, limit):
                n = 0
                while av_q and n < limit:
                    s_av, ex_av = av_q[0]
                    k_av = s_av % LT
                    if v_done[0] <= k_av and s_av < 16:
                        break
                    av_q.pop(0)
                    emit_av(s_av, ex_av)
                    if k_av == LT - 1:
                        emit_norm(s_av // LT)
                    n += 1

            sides_next = emit_scores(0)
            for s in range(NSTEPS):
                for due, vc, fn in list(fillers):
                    if due <= s:
                        fillers.remove((due, vc, fn))
                        fn()
                        v_done[0] += vc
                    else:
                        break
                ex = emit_exp(sides_next)
                if s + 1 < NSTEPS:
                    sides_next = emit_scores(s + 1)
                emit_pa(s, ex)
                av_q.append((s, ex))
                if s >= LAG:
                    pop_avs(s, 2 if s >= 40 else 1)
            while fillers:
                due, vc, fn = fillers.pop(0)
                fn()
                v_done[0] += vc
            pop_avs(NSTEPS, len(av_q))

            _psB_stack.close()
            _psS_stack.close()

            # ================= fc tail: lcp=1 =========================
            # psS's 4 banks are free now: wide [128,1024] chunks,
            # whole-chunk evacuations alternating ScalarE/VectorE (the
            # two run concurrently on different chunks), per-et 256KB
            # ships so the final drain is short
            with tc.tile_pool(name="psC", bufs=3, space="PSUM") as psC:
                for et in range(ET):
                    fp = psC.tile([128, 1024], f32, tag="fc", name=f"fc{et}")
                    for half in range(2):
                        for pair in range(NPAIRS):
                            nc.tensor.matmul(
                                fp[:, ts(half, 512)],
                                wo_sb[:, pair, et, :],
                                outTP_sb[:, pair, ds(1024 + half * 512, 512)],
                                start=(pair == 0),
                                stop=(pair == NPAIRS - 1),
                            )
                    ob = xT_sb[:, et, ts(1, 1024)]
                    if et % 2 == 0:
                        nc.scalar.copy(ob, fp)
                    else:
                        nc.vector.tensor_copy(ob, fp)
                    nc.sync.dma_start(
                        out=out_ap[:, ds(et, 1), ts(1, 1024)],
                        in_=xT_sb[:, ds(et, 1), ts(1, 1024)],
                    )

            _psA2_stack.close()

    nc.compile()
    return nc


def get_nc():
    global _nc_cache
    if _nc_cache is None:
        _nc_cache = build_nc()
    return _nc_cache


def make_core_inputs(x, Wq, Wk, Wv, Wo, bo):
    """Build the 8 per-core input maps from the full-size inputs."""
    x = np.asarray(x, F32)
    Wq = np.asarray(Wq, F32)
    Wk = np.asarray(Wk, F32)
    Wv = np.asarray(Wv, F32)
    Wo = np.asarray(Wo, F32)
    bo = np.asarray(bo, F32)

    xT_b = [np.ascontiguousarray(x[n].T).astype(BF16) for n in range(NB)]

    in_maps = []
    for c in range(NCORES):
        n, g = divmod(c, HPG)
        heads = [g * HPG + i for i in range(HPG)]

        wqk = np.empty((4, EMBED, 128), F32)
        for j in range(4):
            pair, qk = divmod(j, 2)
            hA = heads[2 * pair]
            hB = heads[2 * pair + 1]
            W = Wq if qk == 0 else Wk
            wqk[j, :, 0:HD] = W[hA * HD : (hA + 1) * HD, :].T
            wqk[j, :, HD:128] = W[hB * HD : (hB + 1) * HD, :].T

        wv = np.concatenate(
            [Wv[h * HD : (h + 1) * HD, :].T for h in heads], axis=1
        )  # [1024, 256]

        wo = np.empty((NPAIRS, ET, 128, 128), F32)
        for pair in range(NPAIRS):
            hA = heads[2 * pair]
            hB = heads[2 * pair + 1]
            for et in range(ET):
                blk = Wo[et * 128 : (et + 1) * 128, :]
                wo[pair, et, 0:HD, :] = blk[:, hA * HD : (hA + 1) * HD].T
                wo[pair, et, HD:128, :] = blk[:, hB * HD : (hB + 1) * HD].T

        # partition-major relayouts: [p, ...] with contiguous per-p rows
        wqk_t = np.ascontiguousarray(
            wqk.reshape(4, ET, 128, 128).transpose(2, 0, 1, 3)
        )
        wv_t = np.ascontiguousarray(
            wv.reshape(ET, 128, HPG * HD).transpose(1, 0, 2)
        )
        wo_t = np.ascontiguousarray(wo.transpose(2, 0, 1, 3))

        in_maps.append(
            {
                "xT": xT_b[n],
                "wqk": wqk_t.astype(BF16),
                "wv": wv_t.astype(BF16),
                "wo": wo_t.astype(BF16),
            }
        )
    return in_maps


def combine_outputs(results, bo):
    """Sum the per-core fc_out partials, add bias, transpose to [N, L, E]."""
    out = np.empty((NB, L, EMBED), F32)
    for n in range(NB):
        acc = results[n * HPG]["out"].astype(F32)
        for g in range(1, HPG):
            acc = acc + results[n * HPG + g]["out"].astype(F32)
        out[n] = acc.T + np.asarray(bo, F32)
    return out


def kernel(x, Wq, Wk, Wv, Wo, bo):
    global LAST_EXEC_TIME_NS, LAST_RESULTS
    nc = get_nc()
    in_maps = make_core_inputs(x, Wq, Wk, Wv, Wo, bo)
    trace = bool(os.environ.get("KERNEL_TRACE"))
    kw = {}
    if trace:
        kw["trace"] = True
        kw["trace_cores"] = list(range(NCORES))
    res = run_bass_kernel_spmd(nc, in_maps, list(range(NCORES)), **kw)
    LAST_EXEC_TIME_NS = res.exec_time_ns
    LAST_RESULTS = res
    return combine_outputs(res.results, bo)
